# revision 43
# baseline (speedup 1.0000x reference)
"""3-layer GCN (nn_GAT_20899310863186) on 8 TRN2 NeuronCores via Bass/Tile.

Strategy (per sharding hint): nodes are row-sharded 6250/core; edges are
partitioned by destination owner and sorted by (src-half, dst-window, src).
Per layer, per core:
  1. dense part on own rows (x @ W1 for L1), scale rows by dis (=1/sqrt(deg))
     and AllGather the scaled activation table in bf16 so every core holds the
     full [50000, F] gather source in DRAM,
  2. segment-sum over local edges = dma_gather of 128-edge blocks (bf16,
     256B rows) + one-hot matmul accumulated into a PSUM window of 128
     destination nodes.  The one-hot matrices are built ON-CHIP: one batched
     DVE is_equal(iota, dstw) per 16 blocks (tensor_tensor never enters the
     DVE 2-port perf mode, so it does not block SWDGE descriptor generation).
     dis_src is folded into the table; dis_dst is applied in the epilogue, so
     the one-hot entries are exactly 1.0.
  3. epilogue per window: dis*psum + dis^2*own + bias (+ relu), weight matmul
     via PE-transpose where the layer applies W after the aggregation (L2,L3).
Layer algebra: L1 aggregates x@W1; L2 uses A(h1 W2) = (A h1) W2; L3 likewise,
so gather elements stay >=256B and W2/W3 apply post-aggregation on own rows.

int16 gather indices only reach 32767, so the node table is split in two
25000-row halves; lo and hi edge streams are consumed per window into ONE
psum accumulation chain.  Block counts are maxed across cores so all 8 cores
run one SPMD program.  Edges within each (half, window) group are sorted by
src so gather descriptors walk ascending addresses.
"""

import sys

sys.path.insert(0, "/opt/trn_rl_repo")

import numpy as np
import ml_dtypes

import concourse.bacc as bacc
import concourse.mybir as mybir
import concourse.tile as tile
from concourse import library_config
from concourse.bass_utils import run_bass_kernel_spmd

BF16 = ml_dtypes.bfloat16

N, P = 50000, 8
NSH = N // P                 # 6250 nodes per core
F_IN, H1, H2, C = 256, 128, 64, 16
WIN = (NSH + 127) // 128     # 49 destination windows per core
NPAD = WIN * 128             # 6272
HALF = N // 2                # int16 index range split
CHUNK = 4                    # gather blocks per dma_gather call (ucode ring caps at ~1024 idxs)
EQB = 16                     # one-hot blocks built per DVE is_equal op


def _preprocess(edge_index):
    src = np.asarray(edge_index[0]).astype(np.int64)
    dst = np.asarray(edge_index[1]).astype(np.int64)
    E = src.shape[0]

    deg = (1.0 + np.bincount(dst, minlength=N)).astype(np.float32)
    dis = (1.0 / np.sqrt(deg)).astype(np.float32)

    core = dst // NSH
    dstloc = dst - core * NSH
    win = dstloc >> 7
    dstw = (dstloc & 127).astype(np.float32)
    src_core = src // NSH
    src_i = src - src_core * NSH
    prow = src_core * NPAD + src_i            # padded table row
    half = (src_core >= P // 2).astype(np.int64)
    loc_src = (prow - half * (P // 2) * NPAD).astype(np.int16)

    cnt = np.zeros((P, 2, WIN), np.int64)
    np.add.at(cnt, (core, half, win), 1)
    B = np.maximum(1, -(-cnt.max(axis=0) // 128))       # [2, WIN] blocks
    Blo, Bhi = int(B[0].sum()), int(B[1].sum())
    BTOT = Blo + Bhi

    blk_base = np.zeros((2, WIN), np.int64)
    blk_base[0] = np.cumsum(B[0]) - B[0]
    blk_base[1] = np.cumsum(B[1]) - B[1]

    order = np.lexsort((src, win, half, core))
    key = (core * 2 + half) * WIN + win
    ks = key[order]
    starts = np.r_[0, np.flatnonzero(np.diff(ks)) + 1]
    gmark = np.zeros(E, np.int64)
    gmark[starts] = 1
    grp = np.cumsum(gmark) - 1
    rank = np.arange(E) - starts[grp]

    c_s, h_s, w_s = core[order], half[order], win[order]
    slot = blk_base[h_s, w_s] * 128 + rank               # within half-stream

    idx_lo = np.zeros((P, Blo * 128), np.int16)
    idx_hi = np.zeros((P, Bhi * 128), np.int16)
    # padded slots: idx 0 with dstw sentinel 999 -> is_equal never fires
    dstw_s = np.full((P, BTOT * 128), 999.0, np.float32)

    lo = h_s == 0
    idx_lo[c_s[lo], slot[lo]] = loc_src[order][lo]
    idx_hi[c_s[~lo], slot[~lo]] = loc_src[order][~lo]
    gslot = np.where(lo, slot, Blo * 128 + slot)
    dstw_s[c_s, gslot] = dstw[order]

    def wrap_idx(a):
        n = a.shape[1]
        w = a.reshape(P, n // 16, 16).transpose(0, 2, 1)
        return np.ascontiguousarray(np.tile(w, (1, 8, 1)))

    idx_lo_w = wrap_idx(idx_lo)
    idx_hi_w = wrap_idx(idx_hi)
    # per-(slot, block) dst position, [P, 128 slots, BTOT] in bf16
    dw_w = np.ascontiguousarray(
        dstw_s.reshape(P, BTOT, 128).transpose(0, 2, 1)).astype(BF16)

    d2 = np.zeros((P, NPAD), np.float32)
    d2[:, :NSH] = (dis * dis).reshape(P, NSH)
    d2_w = np.ascontiguousarray(d2.reshape(P, WIN, 128).transpose(0, 2, 1))
    ds = np.zeros((P, NPAD), np.float32)
    ds[:, :NSH] = dis.reshape(P, NSH)
    ds_w = np.ascontiguousarray(ds.reshape(P, WIN, 128).transpose(0, 2, 1))

    return {
        "B": B, "idx_lo": idx_lo_w, "idx_hi": idx_hi_w,
        "dstw": dw_w, "d2": d2_w, "disw": ds_w,
    }


def _build(B):
    f32, bf16, i16 = mybir.dt.float32, mybir.dt.bfloat16, mybir.dt.int16
    AO = mybir.AluOpType
    AF = mybir.ActivationFunctionType
    Blo, Bhi = int(B[0].sum()), int(B[1].sum())
    BTOT = Blo + Bhi

    nc = bacc.Bacc("TRN2", num_devices=P, num_swdge_queues=4, dynamic_dma_scratch_size=32768)

    xT_d = nc.dram_tensor("xT", [F_IN, NPAD], bf16, kind="ExternalInput")
    w1_d = nc.dram_tensor("W1", [F_IN, H1], bf16, kind="ExternalInput")
    w2_d = nc.dram_tensor("W2", [H1, H2], f32, kind="ExternalInput")
    w3_d = nc.dram_tensor("W3", [H2, C], f32, kind="ExternalInput")
    b1_d = nc.dram_tensor("b1r", [128, H1], f32, kind="ExternalInput")
    b2_d = nc.dram_tensor("b2r", [128, H2], f32, kind="ExternalInput")
    b3_d = nc.dram_tensor("b3r", [128, C], f32, kind="ExternalInput")
    d2_d = nc.dram_tensor("dis2", [128, WIN], f32, kind="ExternalInput")
    ds_d = nc.dram_tensor("disw", [128, WIN], f32, kind="ExternalInput")
    id_d = nc.dram_tensor("ident", [128, 128], f32, kind="ExternalInput")
    il_d = nc.dram_tensor("idxlo", [128, Blo * 8], i16, kind="ExternalInput")
    ih_d = nc.dram_tensor("idxhi", [128, Bhi * 8], i16, kind="ExternalInput")
    dw_d = nc.dram_tensor("dstw", [128, BTOT], bf16, kind="ExternalInput")
    io_d = nc.dram_tensor("iota", [128, 128], bf16, kind="ExternalInput")
    out_d = nc.dram_tensor("out", [NSH, C], f32, kind="ExternalOutput")

    from contextlib import ExitStack
    with tile.TileContext(nc) as tc, ExitStack() as est:
        nc.gpsimd.load_library(library_config.mlp)
        with (
            tc.tile_pool(name="const", bufs=1) as const,
            tc.tile_pool(name="dram", bufs=1, space="DRAM") as dram,
            tc.tile_pool(name="xp", bufs=8) as xp,
            tc.tile_pool(name="gat", bufs=12) as gat,
            tc.tile_pool(name="eqa", bufs=3) as eqa,
            tc.tile_pool(name="eqb", bufs=3) as eqb,
            tc.tile_pool(name="tmp", bufs=6) as tmp,
            tc.tile_pool(name="hbp", bufs=4) as hbp,
            tc.tile_pool(name="pagg", bufs=4, space="PSUM") as pagg,
            tc.tile_pool(name="ptr", bufs=2, space="PSUM") as ptr,
            tc.tile_pool(name="pww", bufs=2, space="PSUM") as pww,
        ):
            ag_in = [
                dram.tile([NPAD, 128], bf16, name=f"agin{l}") for l in range(3)
            ]
            ag_out = [
                dram.tile([P * NPAD, 128], bf16, addr_space="Shared",
                          name=f"agout{l}")
                for l in range(3)
            ]

            # constants / schedule
            w1_t = const.tile([128, 2, H1], bf16)
            nc.sync.dma_start(w1_t[:], w1_d[:].rearrange("(k p) h -> p k h", p=128))
            w2_t = const.tile([128, H2], f32)
            nc.sync.dma_start(w2_t[:], w2_d[:])
            w3_t = const.tile([H2, C], f32)
            nc.sync.dma_start(w3_t[:], w3_d[:])
            b1_t = const.tile([128, H1], f32)
            nc.sync.dma_start(b1_t[:], b1_d[:])
            b2_t = const.tile([128, H2], f32)
            nc.sync.dma_start(b2_t[:], b2_d[:])
            b3_t = const.tile([128, C], f32)
            nc.sync.dma_start(b3_t[:], b3_d[:])
            d2_t = const.tile([128, WIN], f32)
            nc.sync.dma_start(d2_t[:], d2_d[:])
            ds_t = const.tile([128, WIN], f32)
            nc.sync.dma_start(ds_t[:], ds_d[:])
            idn_t = const.tile([128, 128], f32)
            nc.sync.dma_start(idn_t[:], id_d[:])
            il_t = const.tile([128, Blo * 8], i16)
            nc.scalar.dma_start(il_t[:], il_d[:])
            ih_t = const.tile([128, Bhi * 8], i16)
            nc.scalar.dma_start(ih_t[:], ih_d[:])
            dw_t = const.tile([128, BTOT], bf16)
            nc.scalar.dma_start(dw_t[:], dw_d[:])
            io_t = const.tile([128, 128], bf16)
            nc.sync.dma_start(io_t[:], io_d[:])
            zero_t = const.tile([128, 128], f32)
            nc.vector.memset(zero_t[:], 0.0)

            stage = const.tile([128, WIN, 128], bf16)
            oput = const.tile([128, WIN, C], f32)
            xbig = const.tile([128, 2, NPAD], bf16)
            nc.sync.dma_start(
                xbig[:], xT_d[:].rearrange("(k p) n -> p k n", p=128))
            A1f = const.tile([128, WIN * H1], f32)
            h1f = const.tile([128, WIN * H1], f32)
            h2f = const.tile([128, WIN * H2], f32)

            def ag_rows(w):
                return min(128, NSH - w * 128)

            def table_flush(layer):
                nc.sync.dma_start(
                    ag_in[layer][:].rearrange("(w p) f -> p w f", p=128),
                    stage[:],
                )

            def all_gather(layer):
                nc.gpsimd.collective_compute(
                    "AllGather",
                    AO.bypass,
                    replica_groups=[list(range(P))],
                    ins=[ag_in[layer].opt()],
                    outs=[ag_out[layer].opt()],
                )

            # ---- phase A1 = x @ W1 on own rows; table rows = dis * A1 ----
            for w in range(WIN):
                ps = pagg.tile([128, H1], f32, tag="pagg", name="psA1")
                for k in range(2):
                    nc.tensor.matmul(
                        ps[:], xbig[:, k, w * 128:(w + 1) * 128], w1_t[:, k, :],
                        start=(k == 0), stop=(k == 1)
                    )
                nc.scalar.copy(A1f[:, w * H1:(w + 1) * H1], ps[:])
                nc.scalar.activation(
                    stage[:, w, :], ps[:], AF.Copy, scale=ds_t[:, w:w + 1]
                )
            table_flush(0)

            def run_pass(layer, out_cb):
                bw = [[int(x) for x in B[h]] for h in range(2)]
                q_state = {"q": layer % 4}
                streams = []
                for h in range(2):
                    streams.append({
                        "src": ag_out[layer][
                            (P // 2) * NPAD * h:(P // 2) * NPAD * (h + 1), :],
                        "idx": il_t if h == 0 else ih_t,
                        "nblk": sum(bw[h]),
                        "blk0": 0 if h == 0 else Blo,
                        "gt": None, "gbase": 0, "gnb": 0,
                        "eq": None, "ebase": 0, "enb": 0,
                        "pool": eqa if h == 0 else eqb,
                    })

                def g_slice(st, b):
                    if st["gt"] is None or b >= st["gbase"] + st["gnb"]:
                        nb = min(CHUNK, st["nblk"] - b)
                        q = q_state["q"]
                        t = gat.tile([128, CHUNK, 128], bf16, tag="gat", name="gt")
                        nc.gpsimd.dma_gather(
                            t[:, :nb, :], st["src"], st["idx"][:, b * 8:(b + nb) * 8],
                            nb * 128, nb * 128, 128, queue_num=q,
                        )
                        st.update(gt=t, gbase=b, gnb=nb)
                        q_state["q"] = (q + 1) % 4
                    if st["eq"] is None or b >= st["ebase"] + st["enb"]:
                        ne = min(EQB, st["nblk"] - b)
                        et = st["pool"].tile([128, EQB * 128], bf16, tag="eq", name="eq")
                        gb = st["blk0"] + b
                        dw_ap = dw_t[:, gb:gb + ne].unsqueeze(-1).broadcast_to(
                            [128, ne, 128])
                        io_ap = io_t[:].unsqueeze(1).broadcast_to([128, ne, 128])
                        nc.vector.tensor_tensor(
                            et[:, :ne * 128].rearrange("p (k d) -> p k d", d=128),
                            io_ap, dw_ap, AO.is_equal,
                        )
                        st.update(eq=et, ebase=b, enb=ne)
                    return (st["gt"][:, b - st["gbase"], :],
                            st["eq"][:, (b - st["ebase"]) * 128:
                                     (b - st["ebase"] + 1) * 128])

                for w in range(WIN):
                    ps = pagg.tile([128, 128], f32, tag="pagg", name="psW")
                    nblk = bw[0][w] + bw[1][w]
                    j = 0
                    for h in range(2):
                        st = streams[h]
                        base = sum(bw[h][:w])
                        for k in range(bw[h][w]):
                            g, pm = g_slice(st, base + k)
                            nc.tensor.matmul(
                                ps[:], pm, g, start=(j == 0), stop=(j == nblk - 1)
                            )
                            j += 1
                    out_cb(w, ps)

            def l1_cb(w, ps):
                wsl = slice(w * H1, (w + 1) * H1)
                v = tmp.tile([128, H1], f32, tag="tA", name="v1")
                nc.vector.scalar_tensor_tensor(
                    v[:], A1f[:, wsl], d2_t[:, w:w + 1], b1_t[:], AO.mult, AO.add
                )
                u = tmp.tile([128, H1], f32, tag="tB", name="u1")
                nc.vector.scalar_tensor_tensor(
                    u[:], ps[:], ds_t[:, w:w + 1], v[:], AO.mult, AO.add
                )
                nc.scalar.activation(h1f[:, wsl], u[:], AF.Relu)
                nc.scalar.activation(
                    stage[:, w, :], h1f[:, wsl], AF.Copy, scale=ds_t[:, w:w + 1]
                )
                if w == WIN - 1:
                    table_flush(1)

            def l2_cb(w, ps):
                wsl = slice(w * H1, (w + 1) * H1)
                w64 = slice(w * H2, (w + 1) * H2)
                vh = tmp.tile([128, H1], f32, tag="tA", name="vh2")
                nc.vector.scalar_tensor_tensor(
                    vh[:], h1f[:, wsl], d2_t[:, w:w + 1], zero_t[:], AO.mult, AO.add
                )
                u = tmp.tile([128, H1], f32, tag="tB", name="u2")
                nc.vector.scalar_tensor_tensor(
                    u[:], ps[:], ds_t[:, w:w + 1], vh[:], AO.mult, AO.add
                )
                pt = ptr.tile([128, 128], f32, tag="ptr", name="pt2")
                nc.tensor.transpose(pt[:], u[:], idn_t[:])
                uT = tmp.tile([128, 128], f32, tag="tC", name="uT2")
                nc.scalar.copy(uT[:], pt[:])
                pw = pww.tile([128, H2], f32, tag="pw", name="pw2")
                nc.tensor.matmul(pw[:], uT[:], w2_t[:])
                v = tmp.tile([128, H2], f32, tag="tD", name="v2")
                nc.vector.tensor_tensor(v[:], pw[:], b2_t[:], AO.add)
                nc.scalar.activation(h2f[:, w64], v[:], AF.Relu)
                nc.vector.memset(stage[:, w, H2:], 0.0)
                nc.scalar.activation(
                    stage[:, w, :H2], h2f[:, w64], AF.Copy,
                    scale=ds_t[:, w:w + 1]
                )
                if w == WIN - 1:
                    table_flush(2)

            def l3_cb(w, ps):
                w64 = slice(w * H2, (w + 1) * H2)
                vh = tmp.tile([128, H2], f32, tag="tD", name="vh3")
                nc.vector.scalar_tensor_tensor(
                    vh[:], h2f[:, w64], d2_t[:, w:w + 1], zero_t[:, :H2],
                    AO.mult, AO.add
                )
                u = tmp.tile([128, H2], f32, tag="tA", name="u3")
                nc.vector.scalar_tensor_tensor(
                    u[:], ps[:, :H2], ds_t[:, w:w + 1], vh[:], AO.mult, AO.add
                )
                pt = ptr.tile([128, 128], f32, tag="ptr", name="pt3")
                nc.tensor.transpose(pt[:H2, :], u[:], idn_t[:])
                vT = tmp.tile([128, 128], f32, tag="tC", name="vT3")
                nc.scalar.copy(vT[:H2, :], pt[:H2, :])
                po = pww.tile([128, C], f32, tag="pw", name="po3")
                nc.tensor.matmul(po[:], vT[:H2, :], w3_t[:])
                nc.vector.tensor_tensor(oput[:, w, :], po[:], b3_t[:], AO.add)
                if w == WIN - 1:
                    nc.sync.dma_start(
                        out_d[:48 * 128, :].rearrange("(w p) c -> p w c", p=128),
                        oput[:, :48, :],
                    )
                    nc.sync.dma_start(
                        out_d[48 * 128:, :], oput[:NSH - 48 * 128, 48, :])

            all_gather(0)
            run_pass(0, l1_cb)
            all_gather(1)
            run_pass(1, l2_cb)
            all_gather(2)
            run_pass(2, l3_cb)

    nc.compile()
    return nc


_CACHE = {}
_IOTA = np.ascontiguousarray(
    np.broadcast_to(np.arange(128, dtype=np.float32), (128, 128)).astype(BF16)
)
_IDENT = np.eye(128, dtype=np.float32)


def kernel(**inputs):
    x = np.asarray(inputs["x"], dtype=np.float32)
    ei = np.asarray(inputs["edge_index"])
    W1 = np.asarray(inputs["W1"], dtype=np.float32)
    b1 = np.asarray(inputs["b1"], dtype=np.float32)
    W2 = np.asarray(inputs["W2"], dtype=np.float32)
    b2 = np.asarray(inputs["b2"], dtype=np.float32)
    W3 = np.asarray(inputs["W3"], dtype=np.float32)
    b3 = np.asarray(inputs["b3"], dtype=np.float32)

    key = hash(ei.tobytes())
    if key not in _CACHE:
        pre = _preprocess(ei)
        nc = _build(pre["B"])
        _CACHE[key] = (nc, pre)
    nc, pre = _CACHE[key]

    in_maps = []
    for c in range(P):
        xT = np.zeros((F_IN, NPAD), BF16)
        xT[:, :NSH] = x[c * NSH:(c + 1) * NSH].T
        in_maps.append({
            "xT": xT,
            "W1": W1.astype(BF16), "W2": W2, "W3": W3,
            "b1r": np.ascontiguousarray(np.broadcast_to(b1, (128, H1))),
            "b2r": np.ascontiguousarray(np.broadcast_to(b2, (128, H2))),
            "b3r": np.ascontiguousarray(np.broadcast_to(b3, (128, C))),
            "dis2": pre["d2"][c],
            "disw": pre["disw"][c],
            "ident": _IDENT,
            "iota": _IOTA,
            "idxlo": pre["idx_lo"][c],
            "idxhi": pre["idx_hi"][c],
            "dstw": pre["dstw"][c],
        })

    res = run_bass_kernel_spmd(nc, in_maps, core_ids=list(range(P)))
    out = np.concatenate([res.results[c]["out"] for c in range(P)], axis=0)
    return np.ascontiguousarray(out, dtype=np.float32)


# revision 44
# speedup vs baseline: 1.0173x; 1.0173x over previous
"""3-layer GCN (nn_GAT_20899310863186) on 8 TRN2 NeuronCores via Bass/Tile.

Strategy (per sharding hint): nodes are row-sharded 6250/core; edges are
partitioned by destination owner and sorted by (src-half, dst-window, src).
Per layer, per core:
  1. dense part on own rows (x @ W1 for L1), scale rows by dis (=1/sqrt(deg))
     and AllGather the scaled activation table in bf16 so every core holds the
     full [50000, F] gather source in DRAM,
  2. segment-sum over local edges = dma_gather of 128-edge blocks (bf16,
     256B rows) + one-hot matmul accumulated into a PSUM window of 128
     destination nodes.  The one-hot matrices are built ON-CHIP: one batched
     DVE is_equal(iota, dstw) per 16 blocks (tensor_tensor never enters the
     DVE 2-port perf mode, so it does not block SWDGE descriptor generation).
     dis_src is folded into the table; dis_dst is applied in the epilogue, so
     the one-hot entries are exactly 1.0.
  3. epilogue per window: dis*psum + dis^2*own + bias (+ relu), weight matmul
     via PE-transpose where the layer applies W after the aggregation (L2,L3).
Layer algebra: L1 aggregates x@W1; L2 uses A(h1 W2) = (A h1) W2; L3 likewise,
so gather elements stay >=256B and W2/W3 apply post-aggregation on own rows.

int16 gather indices only reach 32767, so the node table is split in two
25000-row halves; lo and hi edge streams are consumed per window into ONE
psum accumulation chain.  Block counts are maxed across cores so all 8 cores
run one SPMD program.  Edges within each (half, window) group are sorted by
src so gather descriptors walk ascending addresses.
"""

import sys

sys.path.insert(0, "/opt/trn_rl_repo")

import numpy as np
import ml_dtypes

import concourse.bacc as bacc
import concourse.mybir as mybir
import concourse.tile as tile
from concourse import library_config
from concourse.bass_utils import run_bass_kernel_spmd

BF16 = ml_dtypes.bfloat16

N, P = 50000, 8
NSH = N // P                 # 6250 nodes per core
F_IN, H1, H2, C = 256, 128, 64, 16
WIN = (NSH + 127) // 128     # 49 destination windows per core
NPAD = WIN * 128             # 6272
HALF = N // 2                # int16 index range split
CHUNK = 4                    # gather blocks per dma_gather call (ucode ring caps at ~1024 idxs)
EQB = 16                     # one-hot blocks built per DVE is_equal op


def _preprocess(edge_index):
    src = np.asarray(edge_index[0]).astype(np.int64)
    dst = np.asarray(edge_index[1]).astype(np.int64)
    E = src.shape[0]

    deg = (1.0 + np.bincount(dst, minlength=N)).astype(np.float32)
    dis = (1.0 / np.sqrt(deg)).astype(np.float32)

    core = dst // NSH
    dstloc = dst - core * NSH
    win = dstloc >> 7
    dstw = (dstloc & 127).astype(np.float32)
    half = (src >= HALF).astype(np.int64)
    loc_src = (src - half * HALF).astype(np.int16)

    cnt = np.zeros((P, 2, WIN), np.int64)
    np.add.at(cnt, (core, half, win), 1)
    B = np.maximum(1, -(-cnt.max(axis=0) // 128))       # [2, WIN] blocks
    Blo, Bhi = int(B[0].sum()), int(B[1].sum())
    BTOT = Blo + Bhi

    blk_base = np.zeros((2, WIN), np.int64)
    blk_base[0] = np.cumsum(B[0]) - B[0]
    blk_base[1] = np.cumsum(B[1]) - B[1]

    order = np.lexsort((src, win, half, core))
    key = (core * 2 + half) * WIN + win
    ks = key[order]
    starts = np.r_[0, np.flatnonzero(np.diff(ks)) + 1]
    gmark = np.zeros(E, np.int64)
    gmark[starts] = 1
    grp = np.cumsum(gmark) - 1
    rank = np.arange(E) - starts[grp]

    c_s, h_s, w_s = core[order], half[order], win[order]
    slot = blk_base[h_s, w_s] * 128 + rank               # within half-stream

    idx_lo = np.zeros((P, Blo * 128), np.int16)
    idx_hi = np.zeros((P, Bhi * 128), np.int16)
    # padded slots: idx 0 with dstw sentinel 999 -> is_equal never fires
    dstw_s = np.full((P, BTOT * 128), 999.0, np.float32)

    lo = h_s == 0
    idx_lo[c_s[lo], slot[lo]] = loc_src[order][lo]
    idx_hi[c_s[~lo], slot[~lo]] = loc_src[order][~lo]
    gslot = np.where(lo, slot, Blo * 128 + slot)
    dstw_s[c_s, gslot] = dstw[order]

    def wrap_idx(a):
        n = a.shape[1]
        w = a.reshape(P, n // 16, 16).transpose(0, 2, 1)
        return np.ascontiguousarray(np.tile(w, (1, 8, 1)))

    idx_lo_w = wrap_idx(idx_lo)
    idx_hi_w = wrap_idx(idx_hi)
    # per-(slot, block) dst position, [P, 128 slots, BTOT] in bf16
    dw_w = np.ascontiguousarray(
        dstw_s.reshape(P, BTOT, 128).transpose(0, 2, 1)).astype(BF16)

    d2 = np.zeros((P, NPAD), np.float32)
    d2[:, :NSH] = (dis * dis).reshape(P, NSH)
    d2_w = np.ascontiguousarray(d2.reshape(P, WIN, 128).transpose(0, 2, 1))
    ds = np.zeros((P, NPAD), np.float32)
    ds[:, :NSH] = dis.reshape(P, NSH)
    ds_w = np.ascontiguousarray(ds.reshape(P, WIN, 128).transpose(0, 2, 1))

    return {
        "B": B, "idx_lo": idx_lo_w, "idx_hi": idx_hi_w,
        "dstw": dw_w, "d2": d2_w, "disw": ds_w,
    }


def _build(B):
    f32, bf16, i16 = mybir.dt.float32, mybir.dt.bfloat16, mybir.dt.int16
    AO = mybir.AluOpType
    AF = mybir.ActivationFunctionType
    Blo, Bhi = int(B[0].sum()), int(B[1].sum())
    BTOT = Blo + Bhi

    nc = bacc.Bacc("TRN2", num_devices=P, num_swdge_queues=4, dynamic_dma_scratch_size=32768)

    xT_d = nc.dram_tensor("xT", [F_IN, NPAD], bf16, kind="ExternalInput")
    w1_d = nc.dram_tensor("W1", [F_IN, H1], bf16, kind="ExternalInput")
    w2_d = nc.dram_tensor("W2", [H1, H2], f32, kind="ExternalInput")
    w3_d = nc.dram_tensor("W3", [H2, C], f32, kind="ExternalInput")
    b1_d = nc.dram_tensor("b1r", [128, H1], f32, kind="ExternalInput")
    b2_d = nc.dram_tensor("b2r", [128, H2], f32, kind="ExternalInput")
    b3_d = nc.dram_tensor("b3r", [128, C], f32, kind="ExternalInput")
    d2_d = nc.dram_tensor("dis2", [128, WIN], f32, kind="ExternalInput")
    ds_d = nc.dram_tensor("disw", [128, WIN], f32, kind="ExternalInput")
    id_d = nc.dram_tensor("ident", [128, 128], f32, kind="ExternalInput")
    il_d = nc.dram_tensor("idxlo", [128, Blo * 8], i16, kind="ExternalInput")
    ih_d = nc.dram_tensor("idxhi", [128, Bhi * 8], i16, kind="ExternalInput")
    dw_d = nc.dram_tensor("dstw", [128, BTOT], bf16, kind="ExternalInput")
    io_d = nc.dram_tensor("iota", [128, 128], bf16, kind="ExternalInput")
    out_d = nc.dram_tensor("out", [NSH, C], f32, kind="ExternalOutput")

    from contextlib import ExitStack
    with tile.TileContext(nc) as tc, ExitStack() as est:
        nc.gpsimd.load_library(library_config.mlp)
        with (
            tc.tile_pool(name="const", bufs=1) as const,
            tc.tile_pool(name="dram", bufs=1, space="DRAM") as dram,
            tc.tile_pool(name="xp", bufs=8) as xp,
            tc.tile_pool(name="gat", bufs=12) as gat,
            tc.tile_pool(name="eqa", bufs=3) as eqa,
            tc.tile_pool(name="eqb", bufs=3) as eqb,
            tc.tile_pool(name="tmp", bufs=6) as tmp,
            tc.tile_pool(name="hbp", bufs=4) as hbp,
            tc.tile_pool(name="pagg", bufs=4, space="PSUM") as pagg,
            tc.tile_pool(name="ptr", bufs=2, space="PSUM") as ptr,
            tc.tile_pool(name="pww", bufs=2, space="PSUM") as pww,
        ):
            ag_in = [
                dram.tile([NSH, 128], bf16, name=f"agin{l}") for l in range(3)
            ]
            ag_out = [
                dram.tile([N, 128], bf16, addr_space="Shared", name=f"agout{l}")
                for l in range(3)
            ]

            # constants / schedule
            w1_t = const.tile([128, 2, H1], bf16)
            nc.sync.dma_start(w1_t[:], w1_d[:].rearrange("(k p) h -> p k h", p=128))
            w2_t = const.tile([128, H2], f32)
            nc.sync.dma_start(w2_t[:], w2_d[:])
            w3_t = const.tile([H2, C], f32)
            nc.sync.dma_start(w3_t[:], w3_d[:])
            b1_t = const.tile([128, H1], f32)
            nc.sync.dma_start(b1_t[:], b1_d[:])
            b2_t = const.tile([128, H2], f32)
            nc.sync.dma_start(b2_t[:], b2_d[:])
            b3_t = const.tile([128, C], f32)
            nc.sync.dma_start(b3_t[:], b3_d[:])
            d2_t = const.tile([128, WIN], f32)
            nc.sync.dma_start(d2_t[:], d2_d[:])
            ds_t = const.tile([128, WIN], f32)
            nc.sync.dma_start(ds_t[:], ds_d[:])
            idn_t = const.tile([128, 128], f32)
            nc.sync.dma_start(idn_t[:], id_d[:])
            il_t = const.tile([128, Blo * 8], i16)
            nc.scalar.dma_start(il_t[:], il_d[:])
            ih_t = const.tile([128, Bhi * 8], i16)
            nc.scalar.dma_start(ih_t[:], ih_d[:])
            dw_t = const.tile([128, BTOT], bf16)
            nc.scalar.dma_start(dw_t[:], dw_d[:])
            io_t = const.tile([128, 128], bf16)
            nc.sync.dma_start(io_t[:], io_d[:])
            zero_t = const.tile([128, 128], f32)
            nc.vector.memset(zero_t[:], 0.0)

            xbig = const.tile([128, 2, NPAD], bf16)
            nc.sync.dma_start(
                xbig[:], xT_d[:].rearrange("(k p) n -> p k n", p=128))
            A1f = const.tile([128, WIN * H1], f32)
            h1f = const.tile([128, WIN * H1], f32)
            h2f = const.tile([128, WIN * H2], f32)

            def ag_rows(w):
                return min(128, NSH - w * 128)

            def table_write(layer, w, hb):
                r = ag_rows(w)
                nc.sync.dma_start(
                    ag_in[layer][w * 128:w * 128 + r, :], hb[:r, :])

            def all_gather(layer):
                nc.gpsimd.collective_compute(
                    "AllGather",
                    AO.bypass,
                    replica_groups=[list(range(P))],
                    ins=[ag_in[layer].opt()],
                    outs=[ag_out[layer].opt()],
                )

            # ---- phase A1 = x @ W1 on own rows; table rows = dis * A1 ----
            for w in range(WIN):
                ps = pagg.tile([128, H1], f32, tag="pagg", name="psA1")
                for k in range(2):
                    nc.tensor.matmul(
                        ps[:], xbig[:, k, w * 128:(w + 1) * 128], w1_t[:, k, :],
                        start=(k == 0), stop=(k == 1)
                    )
                nc.scalar.copy(A1f[:, w * H1:(w + 1) * H1], ps[:])
                ab = hbp.tile([128, 128], bf16, tag="hb", name="ab")
                nc.scalar.activation(
                    ab[:], ps[:], AF.Copy, scale=ds_t[:, w:w + 1]
                )
                table_write(0, w, ab)

            def run_pass(layer, out_cb):
                bw = [[int(x) for x in B[h]] for h in range(2)]
                q_state = {"q": layer % 4}
                streams = []
                for h in range(2):
                    streams.append({
                        "src": ag_out[layer][HALF * h:HALF * (h + 1), :],
                        "idx": il_t if h == 0 else ih_t,
                        "nblk": sum(bw[h]),
                        "blk0": 0 if h == 0 else Blo,
                        "gt": None, "gbase": 0, "gnb": 0,
                        "eq": None, "ebase": 0, "enb": 0,
                        "pool": eqa if h == 0 else eqb,
                    })

                def g_slice(st, b):
                    if st["gt"] is None or b >= st["gbase"] + st["gnb"]:
                        nb = min(CHUNK, st["nblk"] - b)
                        q = q_state["q"]
                        t = gat.tile([128, CHUNK, 128], bf16, tag="gat", name="gt")
                        nc.gpsimd.dma_gather(
                            t[:, :nb, :], st["src"], st["idx"][:, b * 8:(b + nb) * 8],
                            nb * 128, nb * 128, 128, queue_num=q,
                        )
                        st.update(gt=t, gbase=b, gnb=nb)
                        q_state["q"] = (q + 1) % 4
                    if st["eq"] is None or b >= st["ebase"] + st["enb"]:
                        ne = min(EQB, st["nblk"] - b)
                        et = st["pool"].tile([128, EQB * 128], bf16, tag="eq", name="eq")
                        gb = st["blk0"] + b
                        dw_ap = dw_t[:, gb:gb + ne].unsqueeze(-1).broadcast_to(
                            [128, ne, 128])
                        io_ap = io_t[:].unsqueeze(1).broadcast_to([128, ne, 128])
                        nc.vector.tensor_tensor(
                            et[:, :ne * 128].rearrange("p (k d) -> p k d", d=128),
                            io_ap, dw_ap, AO.is_equal,
                        )
                        st.update(eq=et, ebase=b, enb=ne)
                    return (st["gt"][:, b - st["gbase"], :],
                            st["eq"][:, (b - st["ebase"]) * 128:
                                     (b - st["ebase"] + 1) * 128])

                for w in range(WIN):
                    ps = pagg.tile([128, 128], f32, tag="pagg", name="psW")
                    nblk = bw[0][w] + bw[1][w]
                    j = 0
                    for h in range(2):
                        st = streams[h]
                        base = sum(bw[h][:w])
                        for k in range(bw[h][w]):
                            g, pm = g_slice(st, base + k)
                            nc.tensor.matmul(
                                ps[:], pm, g, start=(j == 0), stop=(j == nblk - 1)
                            )
                            j += 1
                    out_cb(w, ps)

            def l1_cb(w, ps):
                wsl = slice(w * H1, (w + 1) * H1)
                v = tmp.tile([128, H1], f32, tag="tA", name="v1")
                nc.vector.scalar_tensor_tensor(
                    v[:], A1f[:, wsl], d2_t[:, w:w + 1], b1_t[:], AO.mult, AO.add
                )
                u = tmp.tile([128, H1], f32, tag="tB", name="u1")
                nc.vector.scalar_tensor_tensor(
                    u[:], ps[:], ds_t[:, w:w + 1], v[:], AO.mult, AO.add
                )
                nc.scalar.activation(h1f[:, wsl], u[:], AF.Relu)
                hb = hbp.tile([128, 128], bf16, tag="hb", name="hb1")
                nc.scalar.activation(
                    hb[:], h1f[:, wsl], AF.Copy, scale=ds_t[:, w:w + 1]
                )
                table_write(1, w, hb)

            def l2_cb(w, ps):
                wsl = slice(w * H1, (w + 1) * H1)
                w64 = slice(w * H2, (w + 1) * H2)
                vh = tmp.tile([128, H1], f32, tag="tA", name="vh2")
                nc.vector.scalar_tensor_tensor(
                    vh[:], h1f[:, wsl], d2_t[:, w:w + 1], zero_t[:], AO.mult, AO.add
                )
                u = tmp.tile([128, H1], f32, tag="tB", name="u2")
                nc.vector.scalar_tensor_tensor(
                    u[:], ps[:], ds_t[:, w:w + 1], vh[:], AO.mult, AO.add
                )
                pt = ptr.tile([128, 128], f32, tag="ptr", name="pt2")
                nc.tensor.transpose(pt[:], u[:], idn_t[:])
                uT = tmp.tile([128, 128], f32, tag="tC", name="uT2")
                nc.scalar.copy(uT[:], pt[:])
                pw = pww.tile([128, H2], f32, tag="pw", name="pw2")
                nc.tensor.matmul(pw[:], uT[:], w2_t[:])
                v = tmp.tile([128, H2], f32, tag="tD", name="v2")
                nc.vector.tensor_tensor(v[:], pw[:], b2_t[:], AO.add)
                nc.scalar.activation(h2f[:, w64], v[:], AF.Relu)
                hb = hbp.tile([128, 128], bf16, tag="hb", name="hb2")
                nc.vector.memset(hb[:, H2:], 0.0)
                nc.scalar.activation(
                    hb[:, :H2], h2f[:, w64], AF.Copy, scale=ds_t[:, w:w + 1]
                )
                table_write(2, w, hb)

            def l3_cb(w, ps):
                w64 = slice(w * H2, (w + 1) * H2)
                vh = tmp.tile([128, H2], f32, tag="tD", name="vh3")
                nc.vector.scalar_tensor_tensor(
                    vh[:], h2f[:, w64], d2_t[:, w:w + 1], zero_t[:, :H2],
                    AO.mult, AO.add
                )
                u = tmp.tile([128, H2], f32, tag="tA", name="u3")
                nc.vector.scalar_tensor_tensor(
                    u[:], ps[:, :H2], ds_t[:, w:w + 1], vh[:], AO.mult, AO.add
                )
                pt = ptr.tile([128, 128], f32, tag="ptr", name="pt3")
                nc.tensor.transpose(pt[:H2, :], u[:], idn_t[:])
                vT = tmp.tile([128, 128], f32, tag="tC", name="vT3")
                nc.scalar.copy(vT[:H2, :], pt[:H2, :])
                po = pww.tile([128, C], f32, tag="pw", name="po3")
                nc.tensor.matmul(po[:], vT[:H2, :], w3_t[:])
                o = tmp.tile([128, C], f32, tag="tB", name="o3")
                nc.vector.tensor_tensor(o[:], po[:], b3_t[:], AO.add)
                r = ag_rows(w)
                nc.sync.dma_start(out_d[w * 128:w * 128 + r, :], o[:r, :])

            all_gather(0)
            run_pass(0, l1_cb)
            all_gather(1)
            run_pass(1, l2_cb)
            all_gather(2)
            run_pass(2, l3_cb)

    nc.compile()
    return nc


_CACHE = {}
_IOTA = np.ascontiguousarray(
    np.broadcast_to(np.arange(128, dtype=np.float32), (128, 128)).astype(BF16)
)
_IDENT = np.eye(128, dtype=np.float32)


def kernel(**inputs):
    x = np.asarray(inputs["x"], dtype=np.float32)
    ei = np.asarray(inputs["edge_index"])
    W1 = np.asarray(inputs["W1"], dtype=np.float32)
    b1 = np.asarray(inputs["b1"], dtype=np.float32)
    W2 = np.asarray(inputs["W2"], dtype=np.float32)
    b2 = np.asarray(inputs["b2"], dtype=np.float32)
    W3 = np.asarray(inputs["W3"], dtype=np.float32)
    b3 = np.asarray(inputs["b3"], dtype=np.float32)

    key = hash(ei.tobytes())
    if key not in _CACHE:
        pre = _preprocess(ei)
        nc = _build(pre["B"])
        _CACHE[key] = (nc, pre)
    nc, pre = _CACHE[key]

    in_maps = []
    for c in range(P):
        xT = np.zeros((F_IN, NPAD), BF16)
        xT[:, :NSH] = x[c * NSH:(c + 1) * NSH].T
        in_maps.append({
            "xT": xT,
            "W1": W1.astype(BF16), "W2": W2, "W3": W3,
            "b1r": np.ascontiguousarray(np.broadcast_to(b1, (128, H1))),
            "b2r": np.ascontiguousarray(np.broadcast_to(b2, (128, H2))),
            "b3r": np.ascontiguousarray(np.broadcast_to(b3, (128, C))),
            "dis2": pre["d2"][c],
            "disw": pre["disw"][c],
            "ident": _IDENT,
            "iota": _IOTA,
            "idxlo": pre["idx_lo"][c],
            "idxhi": pre["idx_hi"][c],
            "dstw": pre["dstw"][c],
        })

    res = run_bass_kernel_spmd(nc, in_maps, core_ids=list(range(P)))
    out = np.concatenate([res.results[c]["out"] for c in range(P)], axis=0)
    return np.ascontiguousarray(out, dtype=np.float32)
